# revision 6
# baseline (speedup 1.0000x reference)
"""AGCNN_IA Trainium2 kernel: 3x self-att + 1x cross-att + 5x conv-BN-lrelu.

Sharding: data-parallel over batch B=8 across 8 NeuronCores (1 sample/core).
BN batch statistics are AllReduce'd (sum, sumsq per channel) across cores.

Top-k(20) softmax attention is computed WITHOUT indices/gathers:
  - scores for row n are shift-invariant under softmax, so score = 2*G - xx[m]
    (self) or cos-sim (cross)
  - top-24 values per row via 3x DVE max8 + 2x match_replace
  - threshold th = 20th value; W = (score >= th) * exp(score - th); row sum Z
    from the top-20 values directly
  - the aggregation sum_j W[n,j] f[j,:] is a dense matmul; W is transposed
    on the PE by multiplying with diag(1/Z) (normalization fused for free)
"""

import sys

import numpy as np

sys.path.insert(0, "/opt/trn_rl_repo")

B = 8
N = 2048
KTOP = 20
EPS_BN = 1e-5
NCH = N // 128  # 16 row chunks
NB = N // 512  # 4 matmul free-dim blocks
BN_CNT = float(B * N)

_CACHE = {}


def _build():
    import concourse.bass as bass
    import concourse.mybir as mybir
    from concourse import bacc, tile
    from concourse.masks import make_identity

    dt = mybir.dt.float32
    AF = mybir.ActivationFunctionType
    ALU = mybir.AluOpType
    AX = mybir.AxisListType

    nc = bacc.Bacc(None, target_bir_lowering=False, debug=False, num_devices=8)

    x_d = nc.declare_dram_parameter("x", [3, N], dt, isOutput=False)
    y3_d = nc.declare_dram_parameter("y3", [128, N], dt, isOutput=False)
    w_d = {}
    g_d = {}
    b_d = {}
    convs = {1: (6, 64), 2: (128, 64), 3: (128, 128), 4: (256, 256), 5: (512, 512)}
    for i, (ci, co) in convs.items():
        w_d[i] = nc.declare_dram_parameter(f"w{i}", [co, ci], dt, isOutput=False)
        g_d[i] = nc.declare_dram_parameter(f"g{i}", [co], dt, isOutput=False)
        b_d[i] = nc.declare_dram_parameter(f"b{i}", [co], dt, isOutput=False)
    out_d = nc.declare_dram_parameter("out", [512, N], dt, isOutput=True)

    with tile.TileContext(nc) as tc:
        with (
            tc.tile_pool(name="persist", bufs=1) as persist,
            tc.tile_pool(name="scratch", bufs=1) as scratch_pool,
            tc.tile_pool(name="dram", bufs=1, space="DRAM") as dram,
        ):
            ident = persist.tile([128, 128], dt)
            make_identity(nc, ident[:, :])
            ones_col = persist.tile([128, 1], dt)
            nc.vector.memset(ones_col[:, :], 1.0)
            ones_row = persist.tile([1, 128], dt)
            nc.vector.memset(ones_row[:, :], 1.0)
            eps_t = persist.tile([128, 1], dt)
            nc.vector.memset(eps_t[:, :], EPS_BN)
            # xc: concat buffer [128, 4, N]; ch c of concat = xc[c%128, c//128, :]
            xc = persist.tile([128, 4, N], dt)
            # big elementwise scratch
            scratch = scratch_pool.tile([128, N], dt)

            def col_sumsq_row(src_ap, C, dst_row_ap):
                """dst_row[0, m] = sum_c src[c, m]^2 (via ACT square + PE ones-matmul)."""
                nc.scalar.activation(scratch[0:C, :], src_ap, AF.Square)
                with tc.tile_pool(name="psx", bufs=2, space="PSUM") as psx:
                    for nb in range(NB):
                        ps = psx.tile([1, 512], dt)
                        nc.tensor.matmul(
                            ps[:, :],
                            ones_col[0:C, :],
                            scratch[0:C, nb * 512 : (nb + 1) * 512],
                            start=True,
                            stop=True,
                        )
                        nc.scalar.copy(
                            dst_row_ap[0:1, nb * 512 : (nb + 1) * 512], ps[:, :]
                        )

            def broadcast_row(row_ap, dst_ap):
                """dst[p, m] = row[0, m] for all 128 partitions (PE K=1 matmul)."""
                with tc.tile_pool(name="psb", bufs=2, space="PSUM") as psb:
                    for nb in range(NB):
                        ps = psb.tile([128, 512], dt)
                        nc.tensor.matmul(
                            ps[:, :],
                            ones_row[:, :],
                            row_ap[0:1, nb * 512 : (nb + 1) * 512],
                            start=True,
                            stop=True,
                        )
                        nc.scalar.copy(dst_ap[:, nb * 512 : (nb + 1) * 512], ps[:, :])

            def transpose_to(src_ap, C, dst_tile):
                """src [C, N] -> dst [128, NCH, C] (dst[:, t, :] = src[:, t*128:...].T)."""
                with tc.tile_pool(name="pst", bufs=2, space="PSUM") as pst:
                    for t0 in range(0, NCH, 4):
                        ps = pst.tile([128, 4, C], dt)
                        for t in range(t0, t0 + 4):
                            nc.tensor.transpose(
                                ps[:, t - t0, :],
                                src_ap[:, t * 128 : (t + 1) * 128],
                                ident[0:C, 0:C],
                            )
                        nc.scalar.copy(dst_tile[:, t0 : t0 + 4, :], ps[:, :, :])

            def attention(att_in, C, fT, f_src, rnx_col, get_rhs, agg_dst):
                """Shared top-k attention. Produces agg_dst [C?, N] = W @ f - x.

                att_in: [C(+1), N] query features (aug row included for self-att)
                fT: [128, NCH, Cf] transposed gather-source features
                f_src: [Cf, N] subtract source (= att_in rows for self, x3 for cross)
                rnx_col: [128, NCH] per-row 1/||x_n|| or None (self-att)
                get_rhs(nb): rhs AP [Ck, 512] for the score matmul
                """
                Cf = fT.shape[2]
                with (
                    tc.tile_pool(name="att_sb", bufs=2) as att_sb,
                    tc.tile_pool(name="att_small", bufs=2) as small,
                    tc.tile_pool(name="ps_g", bufs=3, space="PSUM") as ps_g,
                    tc.tile_pool(name="ps_a", bufs=2, space="PSUM") as ps_a,
                ):
                    for i in range(NCH):
                        isl = slice(i * 128, (i + 1) * 128)
                        score = att_sb.tile([128, N], dt, tag="score")
                        # scores
                        for nb in range(NB):
                            ps = ps_g.tile([128, 512], dt, tag="ps")
                            nc.tensor.matmul(
                                ps[:, :],
                                att_in[:, isl],
                                get_rhs(nb),
                                start=True,
                                stop=True,
                            )
                            if rnx_col is None:
                                nc.scalar.copy(
                                    score[:, nb * 512 : (nb + 1) * 512], ps[:, :]
                                )
                            else:
                                nc.scalar.activation(
                                    score[:, nb * 512 : (nb + 1) * 512],
                                    ps[:, :],
                                    AF.Copy,
                                    scale=rnx_col[:, i : i + 1],
                                )
                        # top-24 values
                        t24 = small.tile([128, 32], dt, tag="t24")
                        mr = att_sb.tile([128, N], dt, tag="mr")
                        nc.vector.max(t24[:, 0:8], score[:, :])
                        nc.vector.match_replace(mr[:, :], t24[:, 0:8], score[:, :], -1e30)
                        nc.vector.max(t24[:, 8:16], mr[:, :])
                        nc.vector.match_replace(mr[:, :], t24[:, 8:16], mr[:, :], -1e30)
                        nc.vector.max(t24[:, 16:24], mr[:, :])
                        th = t24[:, 19:20]
                        # exp bias = -row_max (numerical stability, as softmax does)
                        nth = small.tile([128, 1], dt, tag="nth")
                        nc.vector.tensor_scalar_mul(nth[:, :], t24[:, 0:1], -1.0)
                        e20 = small.tile([128, 20], dt, tag="e20")
                        nc.scalar.activation(
                            e20[:, :], t24[:, 0:20], AF.Exp, bias=nth[:, :]
                        )
                        z = small.tile([128, 1], dt, tag="z")
                        nc.vector.reduce_sum(z[:, :], e20[:, :], axis=AX.X)
                        rz = small.tile([128, 1], dt, tag="rz")
                        nc.vector.reciprocal(rz[:, :], z[:, :])
                        # e = exp(score - th); W = (score >= th) * e  (into score)
                        e = att_sb.tile([128, N], dt, tag="e")
                        nc.scalar.activation(e[:, :], score[:, :], AF.Exp, bias=nth[:, :])
                        nc.vector.scalar_tensor_tensor(
                            score[:, :],
                            score[:, :],
                            th,
                            e[:, :],
                            op0=ALU.is_ge,
                            op1=ALU.mult,
                        )
                        # diag(rz); W^T blocks scaled by rz via matmul
                        dg = small.tile([128, 128], dt, tag="dg")
                        nc.vector.tensor_scalar_mul(dg[:, :], ident[:, :], rz[:, :])
                        wt = att_sb.tile([128, NCH, 128], dt, tag="wt")
                        for j0 in range(0, NCH, 4):
                            ps = ps_g.tile([128, 4, 128], dt, tag="ps")
                            for j in range(j0, j0 + 4):
                                nc.tensor.matmul(
                                    ps[:, j - j0, :],
                                    score[:, j * 128 : (j + 1) * 128],
                                    dg[:, :],
                                    start=True,
                                    stop=True,
                                )
                            nc.scalar.copy(wt[:, j0 : j0 + 4, :], ps[:, :, :])
                        # agg^T[d, n] = sum_j fT[j, d] * WT[j, n]
                        pa = ps_a.tile([Cf, 128], dt, tag="pa")
                        for j in range(NCH):
                            nc.tensor.matmul(
                                pa[:, :],
                                fT[:, j, :],
                                wt[:, j, :],
                                start=(j == 0),
                                stop=(j == NCH - 1),
                            )
                        nc.vector.tensor_sub(agg_dst[:, isl], pa[:, :], f_src[:, isl])

            def conv_bn_lrelu(chunks, li, dests):
                """chunks: list of (ap [Ck, N], Ck). dests: list of CO out APs [coP, N]."""
                C_in, C_out = convs[li]
                coP = min(C_out, 128)
                CO = (C_out + 127) // 128
                KC = len(chunks)
                assert sum(c for _, c in chunks) == C_in
                with (
                    tc.tile_pool(name=f"conv{li}", bufs=1) as cp,
                    tc.tile_pool(name=f"convps{li}", bufs=3, space="PSUM") as cps,
                ):
                    w_sb = cp.tile([coP, CO, C_in], dt)
                    nc.sync.dma_start(
                        w_sb[:, :, :],
                        w_d[li].ap().rearrange("(a p) c -> p a c", p=coP),
                    )
                    wt_sb = cp.tile([128, KC, CO, coP], dt)
                    off = 0
                    for kc, (_, Ck) in enumerate(chunks):
                        for co in range(CO):
                            ps = cps.tile([128, 128], dt, tag="wtps")
                            nc.tensor.transpose(
                                ps[0:Ck, 0:coP],
                                w_sb[0:coP, co, off : off + Ck],
                                ident[0:coP, 0:coP],
                            )
                            nc.scalar.copy(wt_sb[0:Ck, kc, co, :], ps[0:Ck, 0:coP])
                        off += Ck
                    y_sb = cp.tile([coP, CO, N], dt)
                    st = cp.tile([coP, CO, 2], dt)
                    sum_parts = cp.tile([coP, CO, NB], dt)
                    for co in range(CO):
                        for nb in range(NB):
                            ps = cps.tile([128, 512], dt, tag="cps")
                            for kc, (cap, Ck) in enumerate(chunks):
                                nc.tensor.matmul(
                                    ps[0:coP, :],
                                    wt_sb[0:Ck, kc, co, :],
                                    cap[:, nb * 512 : (nb + 1) * 512],
                                    start=(kc == 0),
                                    stop=(kc == KC - 1),
                                )
                            nc.scalar.activation(
                                y_sb[:, co, nb * 512 : (nb + 1) * 512],
                                ps[0:coP, :],
                                AF.Copy,
                                accum_out=sum_parts[:, co, nb : nb + 1],
                            )
                        nc.scalar.activation(
                            scratch[0:coP, :],
                            y_sb[:, co, :],
                            AF.Square,
                            accum_out=st[:, co, 1:2],
                        )
                    nc.vector.reduce_sum(st[:, :, 0:1], sum_parts[:, :, :], axis=AX.X)
                    # AllReduce stats across the 8 cores
                    st_in = dram.tile([coP, CO * 2], dt, tag=f"cc_in{li}")
                    st_out = dram.tile([coP, CO * 2], dt, tag=f"cc_out{li}")
                    nc.sync.dma_start(st_in[:, :], st[:, :, :])
                    nc.gpsimd.collective_compute(
                        "AllReduce",
                        ALU.add,
                        replica_groups=[list(range(8))],
                        ins=[st_in[:, :]],
                        outs=[st_out[:, :]],
                    )
                    gst = cp.tile([coP, CO, 2], dt)
                    nc.sync.dma_start(gst[:, :, :], st_out[:, :])
                    # scale/shift from global stats
                    m = cp.tile([coP, CO], dt)
                    ex2 = cp.tile([coP, CO], dt)
                    var = cp.tile([coP, CO], dt)
                    rstd = cp.tile([coP, CO], dt)
                    sc = cp.tile([coP, CO], dt)
                    sh = cp.tile([coP, CO], dt)
                    gg = cp.tile([coP, CO], dt)
                    bb = cp.tile([coP, CO], dt)
                    nc.sync.dma_start(
                        gg[:, :], g_d[li].ap().rearrange("(a p) -> p a", p=coP)
                    )
                    nc.sync.dma_start(
                        bb[:, :], b_d[li].ap().rearrange("(a p) -> p a", p=coP)
                    )
                    nc.vector.tensor_scalar_mul(m[:, :], gst[:, :, 0], 1.0 / BN_CNT)
                    nc.vector.tensor_scalar_mul(ex2[:, :], gst[:, :, 1], 1.0 / BN_CNT)
                    nc.vector.tensor_mul(var[:, :], m[:, :], m[:, :])
                    nc.vector.tensor_sub(var[:, :], ex2[:, :], var[:, :])
                    # rstd = (var+eps)^-0.5 = exp(-0.5*ln(var+eps))
                    nc.scalar.activation(
                        rstd[:, :], var[:, :], AF.Ln, bias=eps_t[0:coP, :]
                    )
                    nc.scalar.activation(rstd[:, :], rstd[:, :], AF.Exp, scale=-0.5)
                    nc.vector.tensor_mul(sc[:, :], gg[:, :], rstd[:, :])
                    nc.vector.tensor_mul(sh[:, :], m[:, :], sc[:, :])
                    nc.vector.tensor_sub(sh[:, :], bb[:, :], sh[:, :])
                    for co in range(CO):
                        nc.scalar.activation(
                            dests[co],
                            y_sb[:, co, :],
                            AF.Lrelu,
                            bias=sh[:, co : co + 1],
                            scale=sc[:, co : co + 1],
                            alpha=0.01,
                        )

            def self_att_layer(att_in, C, li, dests, next_att=None):
                """att_in: [C+1, N] (row C = xx, filled here)."""
                col_sumsq_row(att_in[0:C, :], C, scratch[0:1, :])
                nc.sync.dma_start(att_in[C : C + 1, :], scratch[0:1, :])
                with tc.tile_pool(name=f"sa{li}", bufs=1) as sp:
                    x2d = sp.tile([C + 1, N], dt)
                    # engine ops must start at a 32-aligned partition: memset the
                    # whole range (row C = -1 survives), then overwrite rows 0..C-1
                    nc.vector.memset(x2d[0 : C + 1, :], -1.0)
                    nc.vector.tensor_scalar_mul(x2d[0:C, :], att_in[0:C, :], 2.0)
                    xT = sp.tile([128, NCH, C], dt)
                    transpose_to(att_in[0:C, :], C, xT)
                    agg = sp.tile([C, N], dt)
                    attention(
                        x2d,
                        C,
                        xT,
                        att_in[0:C, :],
                        None,
                        lambda nb: att_in[:, nb * 512 : (nb + 1) * 512],
                        agg,
                    )
                    conv_bn_lrelu([(att_in[0:C, :], C), (agg[:, :], C)], li, dests)
                if next_att is not None:
                    # copy normalized output into next layer's (partition-offset) slot
                    src, dst = next_att
                    nc.sync.dma_start(dst, src)

            # ---------------- Layer 1 ----------------
            att1 = persist.tile([4, N], dt)
            nc.sync.dma_start(att1[0:3, :], x_d[:, :])
            att2 = persist.tile([65, N], dt)
            self_att_layer(
                att1,
                3,
                1,
                [att2[0:64, :]],
                next_att=(att2[0:64, :], xc[0:64, 0, :]),
            )
            # ---------------- Layer 2 ----------------
            att3 = persist.tile([65, N], dt)
            self_att_layer(
                att2,
                64,
                2,
                [att3[0:64, :]],
                next_att=(att3[0:64, :], xc[64:128, 0, :]),
            )
            # ---------------- Layer 3 ----------------
            self_att_layer(att3, 64, 3, [xc[:, 1, :]])
            # ---------------- Layer 4 (cross) ----------------
            x3 = xc[:, 1, :]
            with tc.tile_pool(name="ca", bufs=1) as ca:
                y3_sb = ca.tile([128, N], dt)
                nc.sync.dma_start(y3_sb[:, :], y3_d[:, :])
                # rnx (per-row 1/||x3_n||) in column form
                xx3 = ca.tile([1, N], dt)
                col_sumsq_row(x3, 128, xx3)
                xx3c = ca.tile([128, NCH], dt)
                with tc.tile_pool(name="psr", bufs=2, space="PSUM") as psr:
                    ps = psr.tile([128, NCH], dt)
                    for t in range(NCH):
                        nc.tensor.transpose(
                            ps[:, t : t + 1],
                            xx3[0:1, t * 128 : (t + 1) * 128],
                            ident[0:1, 0:1],
                        )
                    nc.scalar.copy(xx3c[:, :], ps[:, :])
                rnxc = ca.tile([128, NCH], dt)
                nc.scalar.activation(rnxc[:, :], xx3c[:, :], AF.Ln)
                nc.scalar.activation(rnxc[:, :], rnxc[:, :], AF.Exp, scale=-0.5)
                # rny (per-col 1/||y_m||) in row form; yn = y3 * rny
                yy = ca.tile([1, N], dt)
                col_sumsq_row(y3_sb[:, :], 128, yy)
                rny = ca.tile([1, N], dt)
                nc.scalar.activation(rny[:, :], yy[:, :], AF.Ln)
                nc.scalar.activation(rny[:, :], rny[:, :], AF.Exp, scale=-0.5)
                rnyb = ca.tile([128, N], dt)
                broadcast_row(rny, rnyb)
                yn = ca.tile([128, N], dt)
                nc.vector.tensor_mul(yn[:, :], y3_sb[:, :], rnyb[:, :])
                y3T = ca.tile([128, NCH, 128], dt)
                transpose_to(y3_sb[:, :], 128, y3T)
                agg4 = ca.tile([128, N], dt)
                attention(
                    x3,
                    128,
                    y3T,
                    x3,
                    rnxc,
                    lambda nb: yn[:, nb * 512 : (nb + 1) * 512],
                    agg4,
                )
                conv_bn_lrelu(
                    [(x3, 128), (agg4[:, :], 128)], 4, [xc[:, 2, :], xc[:, 3, :]]
                )
            # ---------------- Layer 5 ----------------
            with tc.tile_pool(name="l5", bufs=1) as l5:
                out_sb = l5.tile([128, 4, N], dt)
                conv_bn_lrelu(
                    [(xc[:, c, :], 128) for c in range(4)],
                    5,
                    [out_sb[:, c, :] for c in range(4)],
                )
                nc.sync.dma_start(
                    out_d.ap().rearrange("(a p) n -> p a n", p=128), out_sb[:, :, :]
                )

    nc.finalize()
    return nc


def kernel(**inputs):
    if "nc" not in _CACHE:
        _CACHE["nc"] = _build()
    nc = _CACHE["nc"]
    from concourse.bass_utils import run_bass_kernel_spmd

    names = ["w1", "w2", "w3", "w4", "w5"] + [
        f"{p}{i}" for i in range(1, 6) for p in ("g", "b")
    ]
    in_maps = []
    for b in range(B):
        m = {
            "x": np.ascontiguousarray(inputs["x"][b]),
            "y3": np.ascontiguousarray(inputs["y3"][b]),
        }
        for k in names:
            m[k] = np.ascontiguousarray(inputs[k])
        in_maps.append(m)
    res = run_bass_kernel_spmd(nc, in_maps, core_ids=list(range(B)))
    return np.stack([res.results[b]["out"] for b in range(B)])


# revision 15
# speedup vs baseline: 1.8363x; 1.8363x over previous
"""AGCNN_IA Trainium2 kernel: 3x self-att + 1x cross-att + 5x conv-BN-lrelu.

Sharding: data-parallel over batch B=8 across 8 NeuronCores (1 sample/core).
BN batch statistics are AllReduce'd (sum, sumsq per channel) across cores.

Top-k(20) softmax attention is computed WITHOUT indices/gathers:
  - scores for row n are shift-invariant under softmax, so score = 2*G - xx[m]
    (self) or cos-sim (cross)
  - top-24 values per row via 3x DVE max8 + 2x match_replace
  - threshold th = 20th value; W = (score >= th) * exp(score - th); row sum Z
    from the top-20 values directly
  - the aggregation sum_j W[n,j] f[j,:] is a dense matmul; W is transposed
    on the PE by multiplying with diag(1/Z) (normalization fused for free)
"""

import sys

import numpy as np

sys.path.insert(0, "/opt/trn_rl_repo")

B = 8
N = 2048
KTOP = 20
EPS_BN = 1e-5
NCH = N // 128  # 16 row chunks
NB = N // 512  # 4 matmul free-dim blocks
BN_CNT = float(B * N)

_CACHE = {}


def _build():
    import concourse.bass as bass
    import concourse.mybir as mybir
    from concourse import bacc, tile
    from concourse.masks import make_identity

    dt = mybir.dt.float32
    bf = mybir.dt.bfloat16
    f32r = mybir.dt.float32r
    AF = mybir.ActivationFunctionType
    ALU = mybir.AluOpType
    AX = mybir.AxisListType

    nc = bacc.Bacc(None, target_bir_lowering=False, debug=False, num_devices=8)

    x_d = nc.declare_dram_parameter("x", [3, N], dt, isOutput=False)
    y3_d = nc.declare_dram_parameter("y3", [128, N], dt, isOutput=False)
    w_d = {}
    g_d = {}
    b_d = {}
    convs = {1: (6, 64), 2: (128, 64), 3: (128, 128), 4: (256, 256), 5: (512, 512)}
    for i, (ci, co) in convs.items():
        w_d[i] = nc.declare_dram_parameter(f"w{i}", [co, ci], dt, isOutput=False)
        g_d[i] = nc.declare_dram_parameter(f"g{i}", [co], dt, isOutput=False)
        b_d[i] = nc.declare_dram_parameter(f"b{i}", [co], dt, isOutput=False)
    out_d = nc.declare_dram_parameter("out", [512, N], dt, isOutput=True)

    with tile.TileContext(nc) as tc:
        with (
            tc.tile_pool(name="persist", bufs=1) as persist,
            tc.tile_pool(name="scratch", bufs=1) as scratch_pool,
            tc.tile_pool(name="dram", bufs=1, space="DRAM") as dram,
        ):
            ident = persist.tile([128, 128], dt)
            make_identity(nc, ident[:, :])
            ident_bf = persist.tile([128, 128], bf)
            make_identity(nc, ident_bf[:, :])
            ones_col = persist.tile([128, 1], dt)
            nc.vector.memset(ones_col[:, :], 1.0)
            ones_row = persist.tile([1, 128], dt)
            nc.vector.memset(ones_row[:, :], 1.0)
            eps_t = persist.tile([128, 1], dt)
            nc.vector.memset(eps_t[:, :], EPS_BN)
            # xc: concat buffer [128, 4, N]; ch c of concat = xc[c%128, c//128, :]
            xc = persist.tile([128, 4, N], dt)
            # big elementwise scratch
            scratch = scratch_pool.tile([128, N], dt)

            def col_sumsq_row(src_ap, C, dst_row_ap):
                """dst_row[0, m] = sum_c src[c, m]^2 (via ACT square + PE ones-matmul)."""
                nc.scalar.activation(scratch[0:C, :], src_ap, AF.Square)
                with tc.tile_pool(name="psx", bufs=2, space="PSUM") as psx:
                    for nb in range(NB):
                        ps = psx.tile([1, 512], dt)
                        nc.tensor.matmul(
                            ps[:, :],
                            ones_col[0:C, :],
                            scratch[0:C, nb * 512 : (nb + 1) * 512],
                            start=True,
                            stop=True,
                        )
                        nc.scalar.copy(
                            dst_row_ap[0:1, nb * 512 : (nb + 1) * 512], ps[:, :]
                        )

            def broadcast_row(row_ap, dst_ap):
                """dst[p, m] = row[0, m] for all 128 partitions (PE K=1 matmul)."""
                with tc.tile_pool(name="psb", bufs=2, space="PSUM") as psb:
                    for nb in range(NB):
                        ps = psb.tile([128, 512], dt)
                        nc.tensor.matmul(
                            ps[:, :],
                            ones_row[:, :],
                            row_ap[0:1, nb * 512 : (nb + 1) * 512],
                            start=True,
                            stop=True,
                        )
                        nc.scalar.copy(dst_ap[:, nb * 512 : (nb + 1) * 512], ps[:, :])

            def transpose_to(src_ap, C, dst_tile):
                """src [C, N] -> dst [128, NCH, C] (dst[:, t, :] = src[:, t*128:...].T)."""
                with tc.tile_pool(name="pst", bufs=2, space="PSUM") as pst:
                    for t0 in range(0, NCH, 4):
                        ps = pst.tile([128, 4, C], dt)
                        for t in range(t0, t0 + 4):
                            nc.tensor.transpose(
                                ps[:, t - t0, :],
                                src_ap[:, t * 128 : (t + 1) * 128],
                                ident[0:C, 0:C],
                            )
                        nc.scalar.copy(dst_tile[:, t0 : t0 + 4, :], ps[:, :, :])

            def attention(att_in, C, fT, f_src, rnx_col, get_rhs, agg_dst):
                """Shared top-k attention. Produces agg_dst [C?, N] = W @ f - x.

                att_in: [C(+1), N] query features (aug row included for self-att)
                fT: [128, NCH, Cf] transposed gather-source features (bf16)
                f_src: [Cf, N] subtract source (= att_in rows for self, x3 for cross)
                rnx_col: [128, NCH] per-row 1/||x_n|| or None (self-att)
                get_rhs(nb): rhs AP [Ck, 512] for the score matmul
                """
                Cf = fT.shape[2]
                with (
                    tc.tile_pool(name="att_sb", bufs=2) as att_sb,
                    tc.tile_pool(name="att_sg", bufs=1) as att_sg,
                    tc.tile_pool(name="att_small", bufs=2) as small,
                    tc.tile_pool(name="ps_g", bufs=3, space="PSUM") as ps_g,
                    tc.tile_pool(name="ps_t", bufs=2, space="PSUM") as ps_t,
                    tc.tile_pool(name="ps_a", bufs=2, space="PSUM") as ps_a,
                ):
                    for g in range(NCH // 4):  # groups of 4 row-chunks
                        wt4 = att_sg.tile([128, NCH, 4, 128], bf, tag="wt4")
                        for s in range(4):  # sub-chunk within group
                            i = g * 4 + s
                            isl = slice(i * 128, (i + 1) * 128)
                            score = att_sb.tile([128, N], dt, tag="score")
                            # scores (single-pass fp32 via float32r view)
                            for nb in range(NB):
                                ps = ps_g.tile([128, 512], dt, tag="ps")
                                nc.tensor.matmul(
                                    ps[:, :],
                                    att_in[:, isl],
                                    get_rhs(nb),
                                    start=True,
                                    stop=True,
                                )
                                if rnx_col is None:
                                    nc.scalar.copy(
                                        score[:, nb * 512 : (nb + 1) * 512], ps[:, :]
                                    )
                                else:
                                    nc.scalar.activation(
                                        score[:, nb * 512 : (nb + 1) * 512],
                                        ps[:, :],
                                        AF.Copy,
                                        scale=rnx_col[:, i : i + 1],
                                    )
                            # top-24 values
                            t24 = small.tile([128, 32], dt, tag="t24")
                            mr = att_sg.tile([128, N], dt, tag="mr")
                            nc.vector.max(t24[:, 0:8], score[:, :])
                            nc.vector.match_replace(
                                mr[:, :], t24[:, 0:8], score[:, :], -1e30
                            )
                            nc.vector.max(t24[:, 8:16], mr[:, :])
                            nc.vector.match_replace(
                                mr[:, :], t24[:, 8:16], mr[:, :], -1e30
                            )
                            nc.vector.max(t24[:, 16:24], mr[:, :])
                            th = t24[:, 19:20]
                            # bias = -(row_max + ln Z): exp comes out normalized
                            nth = small.tile([128, 1], dt, tag="nth")
                            nc.vector.tensor_scalar_mul(nth[:, :], t24[:, 0:1], -1.0)
                            e20 = small.tile([128, 20], dt, tag="e20")
                            nc.scalar.activation(
                                e20[:, :], t24[:, 0:20], AF.Exp, bias=nth[:, :]
                            )
                            z = small.tile([128, 1], dt, tag="z")
                            nc.vector.reduce_sum(z[:, :], e20[:, :], axis=AX.X)
                            lnz = small.tile([128, 1], dt, tag="lnz")
                            nc.scalar.activation(lnz[:, :], z[:, :], AF.Ln)
                            bias2 = small.tile([128, 1], dt, tag="bias2")
                            nc.vector.tensor_sub(bias2[:, :], nth[:, :], lnz[:, :])
                            # e = exp(score - mx - lnz); W = (score >= th) * e -> bf16
                            e = att_sb.tile([128, N], dt, tag="e")
                            nc.scalar.activation(
                                e[:, :], score[:, :], AF.Exp, bias=bias2[:, :]
                            )
                            wb = att_sb.tile([128, N], bf, tag="wb")
                            nc.vector.scalar_tensor_tensor(
                                wb[:, :],
                                score[:, :],
                                th,
                                e[:, :],
                                op0=ALU.is_ge,
                                op1=ALU.mult,
                            )
                            # W^T blocks (bf16 PE transpose)
                            for j0 in range(0, NCH, 4):
                                ps = ps_t.tile([128, 4, 128], bf, tag="pst")
                                for j in range(j0, j0 + 4):
                                    nc.tensor.transpose(
                                        ps[:, j - j0, :],
                                        wb[:, j * 128 : (j + 1) * 128],
                                        ident_bf[:, :],
                                    )
                                nc.scalar.copy(wt4[:, j0 : j0 + 4, s, :], ps[:, :, :])
                        # agg^T[d, n] = sum_j fT[j, d] * WT[j, n] over the group
                        gsl = slice(g * 512, (g + 1) * 512)
                        pa = ps_a.tile([Cf, 512], dt, tag="pa")
                        for j in range(NCH):
                            nc.tensor.matmul(
                                pa[:, :],
                                fT[:, j, :],
                                wt4[:, j, :, :],
                                start=(j == 0),
                                stop=(j == NCH - 1),
                            )
                        nc.vector.tensor_sub(agg_dst[:, gsl], pa[:, :], f_src[:, gsl])

            def conv_bn_lrelu(chunks, li, dests):
                """chunks: list of (ap [Ck, N], Ck). dests: list of CO out APs [coP, N]."""
                C_in, C_out = convs[li]
                coP = min(C_out, 128)
                CO = (C_out + 127) // 128
                KC = len(chunks)
                assert sum(c for _, c in chunks) == C_in
                with (
                    tc.tile_pool(name=f"conv{li}", bufs=1) as cp,
                    tc.tile_pool(name=f"convps{li}", bufs=3, space="PSUM") as cps,
                ):
                    w_sb = cp.tile([coP, CO, C_in], dt)
                    nc.sync.dma_start(
                        w_sb[:, :, :],
                        w_d[li].ap().rearrange("(a p) c -> p a c", p=coP),
                    )
                    wt_sb = cp.tile([128, KC, CO, coP], dt)
                    off = 0
                    for kc, (_, Ck) in enumerate(chunks):
                        for co in range(CO):
                            ps = cps.tile([128, 128], dt, tag="wtps")
                            nc.tensor.transpose(
                                ps[0:Ck, 0:coP],
                                w_sb[0:coP, co, off : off + Ck],
                                ident[0:coP, 0:coP],
                            )
                            nc.scalar.copy(wt_sb[0:Ck, kc, co, :], ps[0:Ck, 0:coP])
                        off += Ck
                    y_sb = cp.tile([coP, CO, N], dt)
                    st = cp.tile([coP, CO, 2], dt)
                    sum_parts = cp.tile([coP, CO, NB], dt)
                    for co in range(CO):
                        for nb in range(NB):
                            ps = cps.tile([128, 512], dt, tag="cps")
                            for kc, (cap, Ck) in enumerate(chunks):
                                nc.tensor.matmul(
                                    ps[0:coP, :],
                                    wt_sb[0:Ck, kc, co, :],
                                    cap[:, nb * 512 : (nb + 1) * 512],
                                    start=(kc == 0),
                                    stop=(kc == KC - 1),
                                )
                            nc.scalar.activation(
                                y_sb[:, co, nb * 512 : (nb + 1) * 512],
                                ps[0:coP, :],
                                AF.Copy,
                                accum_out=sum_parts[:, co, nb : nb + 1],
                            )
                        nc.scalar.activation(
                            scratch[0:coP, :],
                            y_sb[:, co, :],
                            AF.Square,
                            accum_out=st[:, co, 1:2],
                        )
                    nc.vector.reduce_sum(st[:, :, 0:1], sum_parts[:, :, :], axis=AX.X)
                    # AllReduce stats across the 8 cores
                    st_in = dram.tile([coP, CO * 2], dt, tag=f"cc_in{li}")
                    st_out = dram.tile([coP, CO * 2], dt, tag=f"cc_out{li}")
                    nc.sync.dma_start(st_in[:, :], st[:, :, :])
                    nc.gpsimd.collective_compute(
                        "AllReduce",
                        ALU.add,
                        replica_groups=[list(range(8))],
                        ins=[st_in[:, :]],
                        outs=[st_out[:, :]],
                    )
                    gst = cp.tile([coP, CO, 2], dt)
                    nc.sync.dma_start(gst[:, :, :], st_out[:, :])
                    # scale/shift from global stats
                    m = cp.tile([coP, CO], dt)
                    ex2 = cp.tile([coP, CO], dt)
                    var = cp.tile([coP, CO], dt)
                    rstd = cp.tile([coP, CO], dt)
                    sc = cp.tile([coP, CO], dt)
                    sh = cp.tile([coP, CO], dt)
                    gg = cp.tile([coP, CO], dt)
                    bb = cp.tile([coP, CO], dt)
                    nc.sync.dma_start(
                        gg[:, :], g_d[li].ap().rearrange("(a p) -> p a", p=coP)
                    )
                    nc.sync.dma_start(
                        bb[:, :], b_d[li].ap().rearrange("(a p) -> p a", p=coP)
                    )
                    nc.vector.tensor_scalar_mul(m[:, :], gst[:, :, 0], 1.0 / BN_CNT)
                    nc.vector.tensor_scalar_mul(ex2[:, :], gst[:, :, 1], 1.0 / BN_CNT)
                    nc.vector.tensor_mul(var[:, :], m[:, :], m[:, :])
                    nc.vector.tensor_sub(var[:, :], ex2[:, :], var[:, :])
                    # rstd = (var+eps)^-0.5 = exp(-0.5*ln(var+eps))
                    nc.scalar.activation(
                        rstd[:, :], var[:, :], AF.Ln, bias=eps_t[0:coP, :]
                    )
                    nc.scalar.activation(rstd[:, :], rstd[:, :], AF.Exp, scale=-0.5)
                    nc.vector.tensor_mul(sc[:, :], gg[:, :], rstd[:, :])
                    nc.vector.tensor_mul(sh[:, :], m[:, :], sc[:, :])
                    nc.vector.tensor_sub(sh[:, :], bb[:, :], sh[:, :])
                    for co in range(CO):
                        nc.scalar.activation(
                            dests[co],
                            y_sb[:, co, :],
                            AF.Lrelu,
                            bias=sh[:, co : co + 1],
                            scale=sc[:, co : co + 1],
                            alpha=0.01,
                        )

            def self_att_layer(att_in, C, li, dests, next_att=None):
                """att_in: [C+1, N] (row C = xx, filled here)."""
                col_sumsq_row(att_in[0:C, :], C, scratch[0:1, :])
                nc.sync.dma_start(att_in[C : C + 1, :], scratch[0:1, :])
                with tc.tile_pool(name=f"sa{li}", bufs=1) as sp:
                    x2d = sp.tile([C + 1, N], dt)
                    # engine ops must start at a 32-aligned partition: memset the
                    # whole range (row C = -1 survives), then overwrite rows 0..C-1
                    nc.vector.memset(x2d[0 : C + 1, :], -1.0)
                    nc.vector.tensor_scalar_mul(x2d[0:C, :], att_in[0:C, :], 2.0)
                    xT = sp.tile([128, NCH, C], bf)
                    transpose_to(att_in[0:C, :], C, xT)
                    agg = sp.tile([C, N], dt)
                    attention(
                        x2d,
                        C,
                        xT,
                        att_in[0:C, :],
                        None,
                        lambda nb: att_in[:, nb * 512 : (nb + 1) * 512],
                        agg,
                    )
                    conv_bn_lrelu([(att_in[0:C, :], C), (agg[:, :], C)], li, dests)
                if next_att is not None:
                    # copy normalized output into next layer's (partition-offset) slot
                    src, dst = next_att
                    nc.sync.dma_start(dst, src)

            # ---------------- Layer 1 ----------------
            att1 = persist.tile([4, N], dt)
            nc.sync.dma_start(att1[0:3, :], x_d[:, :])
            att2 = persist.tile([65, N], dt)
            self_att_layer(
                att1,
                3,
                1,
                [att2[0:64, :]],
                next_att=(att2[0:64, :], xc[0:64, 0, :]),
            )
            # ---------------- Layer 2 ----------------
            att3 = persist.tile([65, N], dt)
            self_att_layer(
                att2,
                64,
                2,
                [att3[0:64, :]],
                next_att=(att3[0:64, :], xc[64:128, 0, :]),
            )
            # ---------------- Layer 3 ----------------
            self_att_layer(att3, 64, 3, [xc[:, 1, :]])
            # ---------------- Layer 4 (cross) ----------------
            x3 = xc[:, 1, :]
            with tc.tile_pool(name="ca", bufs=1) as ca:
                y3_sb = ca.tile([128, N], dt)
                nc.sync.dma_start(y3_sb[:, :], y3_d[:, :])
                # rnx (per-row 1/||x3_n||) in column form
                xx3 = ca.tile([1, N], dt)
                col_sumsq_row(x3, 128, xx3)
                xx3c = ca.tile([128, NCH], dt)
                with tc.tile_pool(name="psr", bufs=2, space="PSUM") as psr:
                    ps = psr.tile([128, NCH], dt)
                    for t in range(NCH):
                        nc.tensor.transpose(
                            ps[:, t : t + 1],
                            xx3[0:1, t * 128 : (t + 1) * 128],
                            ident[0:1, 0:1],
                        )
                    nc.scalar.copy(xx3c[:, :], ps[:, :])
                rnxc = ca.tile([128, NCH], dt)
                nc.scalar.activation(rnxc[:, :], xx3c[:, :], AF.Ln)
                nc.scalar.activation(rnxc[:, :], rnxc[:, :], AF.Exp, scale=-0.5)
                # rny (per-col 1/||y_m||) in row form; yn = y3 * rny
                yy = ca.tile([1, N], dt)
                col_sumsq_row(y3_sb[:, :], 128, yy)
                rny = ca.tile([1, N], dt)
                nc.scalar.activation(rny[:, :], yy[:, :], AF.Ln)
                nc.scalar.activation(rny[:, :], rny[:, :], AF.Exp, scale=-0.5)
                rnyb = ca.tile([128, N], dt)
                broadcast_row(rny, rnyb)
                yn = ca.tile([128, N], dt)
                nc.vector.tensor_mul(yn[:, :], y3_sb[:, :], rnyb[:, :])
                y3T = ca.tile([128, NCH, 128], bf)
                transpose_to(y3_sb[:, :], 128, y3T)
                agg4 = ca.tile([128, N], dt)
                attention(
                    x3,
                    128,
                    y3T,
                    x3,
                    rnxc,
                    lambda nb: yn[:, nb * 512 : (nb + 1) * 512],
                    agg4,
                )
                conv_bn_lrelu(
                    [(x3, 128), (agg4[:, :], 128)], 4, [xc[:, 2, :], xc[:, 3, :]]
                )
            # ---------------- Layer 5 ----------------
            with tc.tile_pool(name="l5", bufs=1) as l5:
                out_sb = l5.tile([128, 4, N], dt)
                conv_bn_lrelu(
                    [(xc[:, c, :], 128) for c in range(4)],
                    5,
                    [out_sb[:, c, :] for c in range(4)],
                )
                nc.sync.dma_start(
                    out_d.ap().rearrange("(a p) n -> p a n", p=128), out_sb[:, :, :]
                )

    nc.finalize()
    return nc


def kernel(**inputs):
    if "nc" not in _CACHE:
        _CACHE["nc"] = _build()
    nc = _CACHE["nc"]
    from concourse.bass_utils import run_bass_kernel_spmd

    names = ["w1", "w2", "w3", "w4", "w5"] + [
        f"{p}{i}" for i in range(1, 6) for p in ("g", "b")
    ]
    in_maps = []
    for b in range(B):
        m = {
            "x": np.ascontiguousarray(inputs["x"][b]),
            "y3": np.ascontiguousarray(inputs["y3"][b]),
        }
        for k in names:
            m[k] = np.ascontiguousarray(inputs[k])
        in_maps.append(m)
    res = run_bass_kernel_spmd(nc, in_maps, core_ids=list(range(B)))
    return np.stack([res.results[b]["out"] for b in range(B)])
